# revision 9
# baseline (speedup 1.0000x reference)
"""LoRA layer kernel for Trainium2: out = (W + A@B) @ x.

Shapes (fp32): W [4096,4096], A [4096,16], B [16,4096], x [4096,8192],
out [4096,8192].

Strategy (tensor-parallel, 8 NeuronCores):
- Shard W and A row-wise (output dim): 512 rows per core. Replicate B, x.
- Per core, on device:
    1. Load W_shard^T (pre-cast fp16 on host) into SBUF in 8 ko-group
       chunks on the scalar-ring HWDGE DMA queue.
    2. delta^T = B^T @ A_shard^T via 32 K=16 fp16 matmuls (fp32 PSUM;
       A/B pre-cast on host); add in place into the fp16 W^T tiles
       (VectorE, fp32 psum operand) giving W'^T.
    3. Stream x (pre-cast fp16 on host) in 16 n-tiles of 512 columns on
       the sync-ring HWDGE queue; per n-tile compute the 512x512 output
       block via 4 (m) x 32 (k) fp16 matmuls accumulating fp32 in PSUM;
       evict to SBUF as fp16 (VectorE) and DMA out on the scalar-ring
       HWDGE queue.
- Host gathers the 8 row-shards and upcasts to fp32.

Engine/queue choices (hardware-measured on trn2, For_i slope timing):
- Both DMA streams on HWDGE rings (x: sync, out+W: scalar). The original
  gpsimd (SWDGE) out-path serialized against the x stream and compute:
  moving it to the scalar HWDGE ring alone recovered ~260us/pass.
- The PSUM->SBUF eviction stores fp16 instead of fp32 (halves the
  out-write bytes): another ~55us/pass. Output rounding to fp16 adds
  ~1e-4 relative error (out elements are O(260); fp16 has 10 mantissa
  bits), total kernel error stays ~4e-4 vs the 2e-2 gate.
- fp16 matmul inputs: the PE runs 16-bit matmuls at 1 column/cycle (4x
  faster than fp32) with fp32 PSUM accumulation. fp8 double-pumping was
  numerically simulated and rejected: raw e4m3 on both operands gives
  3.7e-2 error (gate 2e-2), and the accurate hi+lo-pair variants cost
  the same cycles as fp16.
"""

import numpy as np

import concourse.bacc as bacc
import concourse.mybir as mybir
import concourse.tile as tile
from concourse.bass_utils import run_bass_kernel_spmd

P = 128          # partitions / systolic dim
OUT = 4096
IN = 4096
RANK = 16
NTOK = 8192
NCORES = 8
MSH = OUT // NCORES          # 512 output rows per core
KS = IN // P                 # 32 k-subtiles
MO = MSH // P                # 4 m-subtiles per core
NT = 16                      # n-tiles
NF = NTOK // NT              # 512 columns per n-tile
NG = 8                       # W ko-group chunks
KG = KS // NG                # 4 ko per group


def build_nc(reps=None, startup_in_loop=False):
    """Build the per-core kernel. With reps=N the main loop is wrapped in
    a hardware For_i(0, N) loop (used by test.py's slope timing). With
    startup_in_loop=True the W-load + LoRA-delta prep also moves inside
    the loop, so one loop iteration is exactly the single-shot kernel's
    work (startup + 16 tiles + drain) and the slope needs no startup
    estimate added."""
    nc = bacc.Bacc(None, target_bir_lowering=False, debug=False)

    wT = nc.dram_tensor("wT", [NG, P, KG, MSH], mybir.dt.float16, kind="ExternalInput")
    aT = nc.dram_tensor("aT", [RANK, MSH], mybir.dt.float16, kind="ExternalInput")
    b = nc.dram_tensor("b", [RANK, KS, P], mybir.dt.float16, kind="ExternalInput")
    xh = nc.dram_tensor("xh", [NT, P, KS, NF], mybir.dt.float16, kind="ExternalInput")
    out = nc.dram_tensor("out", [NT, P, MO, NF], mybir.dt.float16, kind="ExternalOutput")

    with tile.TileContext(nc) as tc:
        with (
            tc.tile_pool(name="w16pool", bufs=1) as w16pool,
            tc.tile_pool(name="spool", bufs=1) as spool,
            tc.tile_pool(name="xpool", bufs=3) as xpool,
            tc.tile_pool(name="opool", bufs=2) as opool,
            tc.tile_pool(name="psum", bufs=8, space="PSUM") as psum,
        ):
            aT_sb = spool.tile([RANK, MSH], mybir.dt.float16)
            b_sb = spool.tile([RANK, KS, P], mybir.dt.float16)
            wp16_g = [w16pool.tile([P, KG, MSH], mybir.dt.float16,
                                   tag=f"wp{g}", bufs=1, name=f"wp{g}")
                      for g in range(NG)]

            # W'^T = W^T + (A@B)^T, rounded to fp16.
            # Per ko: psum[p, m] = sum_r b[r, ko*128+p] * aT[r, m]  (K=16 matmul)
            def startup():
                # W chunks split across BOTH HWDGE rings: a single-ring W
                # load queued ~50us of startup serialization (measured);
                # split, it costs ~7us. The sync ring carries x later, but
                # at startup it is otherwise idle.
                nc.scalar.dma_start(aT_sb[:], aT.ap())
                nc.scalar.dma_start(b_sb[:], b.ap())
                for g in range(NG):
                    eng = nc.sync if g >= NG // 2 else nc.scalar
                    eng.dma_start(wp16_g[g][:], wT.ap()[g])
                # dps gets its own 2-bank PSUM tag so the 32 delta tiles do
                # not serialize the main loop's group opens through the pool
                # rotation (hardware-measured: shared rotation made the
                # startup marginal ~79us).
                for ko in range(KS):
                    g, kg = divmod(ko, KG)
                    dps = psum.tile([P, MSH], mybir.dt.float32, tag="dps",
                                    bufs=2, name="dps")
                    nc.tensor.matmul(dps[:], b_sb[:, ko], aT_sb[:], start=True, stop=True)
                    nc.vector.tensor_add(wp16_g[g][:, kg], wp16_g[g][:, kg], dps[:])

            # Main: out[m, n] = sum_k W'[m, k] x[k, n], fp16 inputs, fp32 accum.
            # ko-outer / mo-inner with 4 concurrent PSUM accumulation groups:
            # each x slice xt[:, ko] feeds the 4 m-subtile matmuls back to
            # back (hardware-measured ~50us/pass faster than mo-outer).
            def body():
                for nt in range(NT):
                    xt = xpool.tile([P, KS, NF], mybir.dt.float16, tag="xt", name="xt")
                    nc.sync.dma_start(xt[:], xh.ap()[nt])
                    ot = opool.tile([P, MO, NF], mybir.dt.float16, tag="ot", name="ot")
                    pss = [psum.tile([P, NF], mybir.dt.float32, tag="ps",
                                     bufs=6, name=f"mps{mo}")
                           for mo in range(MO)]
                    for ko in range(KS):
                        g, kg = divmod(ko, KG)
                        for mo in range(MO):
                            nc.tensor.matmul(
                                pss[mo][:],
                                wp16_g[g][:, kg, mo * P:(mo + 1) * P],
                                xt[:, ko],
                                start=(ko == 0),
                                stop=(ko == KS - 1),
                            )
                    for mo in range(MO):
                        nc.vector.tensor_copy(ot[:, mo], pss[mo][:])
                    nc.scalar.dma_start(out.ap()[nt], ot[:])

            if reps is None:
                startup()
                body()
            elif startup_in_loop:
                with tc.For_i(0, reps):
                    startup()
                    body()
            else:
                startup()
                with tc.For_i(0, reps):
                    body()

    nc.compile()
    return nc


_NC_CACHE = None


def _get_nc():
    global _NC_CACHE
    if _NC_CACHE is None:
        _NC_CACHE = build_nc()
    return _NC_CACHE


def prepare_in_maps(x, weight, A, B):
    """Shard + lay out the full inputs into per-core device input maps."""
    x = np.ascontiguousarray(x, dtype=np.float32)
    weight = np.ascontiguousarray(weight, dtype=np.float32)
    A = np.ascontiguousarray(A, dtype=np.float32)
    B = np.ascontiguousarray(B, dtype=np.float32)

    # x [IN, NTOK] -> fp16 [nt, p, ko, j] with k = ko*128+p, n = nt*512+j
    xh = np.ascontiguousarray(
        x.astype(np.float16).reshape(KS, P, NT, NF).transpose(2, 1, 0, 3)
    )
    # B [RANK, IN] -> [r, ko, p]
    b_dev = np.ascontiguousarray(B.astype(np.float16).reshape(RANK, KS, P))

    in_maps = []
    for c in range(NCORES):
        rows = slice(c * MSH, (c + 1) * MSH)
        # W_shard^T [k, m] -> [g, p, kg, m] with k = (g*KG + kg)*P + p
        wT_dev = np.ascontiguousarray(
            weight[rows].T.astype(np.float16).reshape(NG, KG, P, MSH).transpose(0, 2, 1, 3)
        )
        aT_dev = np.ascontiguousarray(A[rows].T.astype(np.float16))
        in_maps.append({"wT": wT_dev, "aT": aT_dev, "b": b_dev, "xh": xh})
    return in_maps


def assemble_output(results):
    """Gather per-core [nt, p, mo, j] fp16 outputs into the full
    [OUT, NTOK] fp32."""
    out = np.empty((OUT, NTOK), dtype=np.float32)
    for c, r in enumerate(results):
        shard = r["out"].astype(np.float32).transpose(2, 1, 0, 3).reshape(MSH, NTOK)
        out[c * MSH:(c + 1) * MSH] = shard
    return out


def kernel(x, weight, A, B):
    nc = _get_nc()
    in_maps = prepare_in_maps(x, weight, A, B)
    res = run_bass_kernel_spmd(nc, in_maps, core_ids=list(range(NCORES)))
    return assemble_output(res.results)


if __name__ == "__main__":
    rng = np.random.default_rng(0)
    x = rng.standard_normal((IN, NTOK), dtype=np.float32)
    weight = rng.standard_normal((OUT, IN), dtype=np.float32)
    A = rng.standard_normal((OUT, RANK), dtype=np.float32)
    B = rng.standard_normal((RANK, IN), dtype=np.float32)
    got = kernel(x, weight, A, B)
    ref = (weight.astype(np.float64) + A.astype(np.float64) @ B.astype(np.float64)) @ x.astype(np.float64)
    err = np.abs(got - ref).max() / np.abs(ref).max()
    rel = np.linalg.norm(got - ref) / np.linalg.norm(ref)
    print("max-rel-to-max err:", err, " norm-rel:", rel)
